# revision 3
# baseline (speedup 1.0000x reference)
"""Tacotron2 location-sensitive attention on 8 TRN2 NeuronCores.

Data-parallel over batch B=128 -> 16 rows per core; params replicated.
Per core (b = 16 batch rows):
  pq    = hidden @ query_W.T                       (PE, tiny)
  loc   = conv1d(aw_cat) @ loc_W.T  -- folded: CW2[a,(c,k)] = loc_W @ conv_W,
          conv done as one matmul over im2col'd input (host-marshalled)
  e     = v . tanh(pq + loc + pm)                  (PE + DVE + ACT)
  w     = softmax(mask(e))                         (DVE + ACT, batch on partitions)
  ctx   = w @ memory                               (PE matvec, streams 64MB/core)
"""

import os
import numpy as np
from contextlib import ExitStack

import concourse.bass as bass
import concourse.bacc as bacc
import concourse.tile as tile
from concourse import mybir, masks
from concourse.bass_utils import run_bass_kernel_spmd

F32 = mybir.dt.float32
ts = bass.ts

N_CORES = 8
B, T = 128, 2048
BP = B // N_CORES          # 16 batch rows per core
RNN, ATT, ENC = 1024, 128, 512
NF, KS, PAD = 32, 31, 15
CK = 2 * KS                # 62 im2col rows
NT = T // 128              # 16 T-chunks of 128
NC4 = T // 512             # 4 T-chunks of 512

_TRACE = os.environ.get("BASS_KERNEL_TRACE", "0") == "1"
LAST_RESULT = None         # BassKernelResults from the most recent run
_NC_CACHE = None


def _build_nc():
    nc = bacc.Bacc("TRN2", target_bir_lowering=False, debug=False,
                   num_devices=N_CORES)

    hiddenT = nc.dram_tensor("hiddenT", [RNN, BP], F32, kind="ExternalInput").ap()
    qWT = nc.dram_tensor("qWT", [RNN, ATT], F32, kind="ExternalInput").ap()
    xs = nc.dram_tensor("xs", [BP, CK, T], F32, kind="ExternalInput").ap()
    cwr = nc.dram_tensor("cwr", [NF, CK], F32, kind="ExternalInput").ap()
    locWT = nc.dram_tensor("locWT", [NF, ATT], F32, kind="ExternalInput").ap()
    vT = nc.dram_tensor("vT", [ATT, 1], F32, kind="ExternalInput").ap()
    pmT = nc.dram_tensor("pmT", [BP, ATT, T], F32, kind="ExternalInput").ap()
    maskadd = nc.dram_tensor("maskadd", [BP, T], F32, kind="ExternalInput").ap()
    mem = nc.dram_tensor("mem", [BP, T, ENC], F32, kind="ExternalInput").ap()

    out_ctx = nc.dram_tensor("out_ctx", [BP, ENC], F32, kind="ExternalOutput").ap()
    out_w = nc.dram_tensor("out_w", [BP, T], F32, kind="ExternalOutput").ap()

    with tile.TileContext(nc) as tc, ExitStack() as ctx:
        const_pool = ctx.enter_context(tc.tile_pool(name="const", bufs=1))
        xs_pool = ctx.enter_context(tc.tile_pool(name="xs", bufs=3))
        pm_pool = ctx.enter_context(tc.tile_pool(name="pm", bufs=3))
        s_pool = ctx.enter_context(tc.tile_pool(name="s", bufs=4))
        th_pool = ctx.enter_context(tc.tile_pool(name="th", bufs=4))
        w_pool = ctx.enter_context(tc.tile_pool(name="w", bufs=1))
        mem_pool = ctx.enter_context(tc.tile_pool(name="mem", bufs=6))
        o_pool = ctx.enter_context(tc.tile_pool(name="o", bufs=4))
        ps_loc_pool = ctx.enter_context(tc.tile_pool(name="psloc", bufs=2, space="PSUM"))
        ps_e_pool = ctx.enter_context(tc.tile_pool(name="pse", bufs=2, space="PSUM"))
        ps_misc_pool = ctx.enter_context(tc.tile_pool(name="psmisc", bufs=2, space="PSUM"))
        ps_ctx_pool = ctx.enter_context(tc.tile_pool(name="psctx", bufs=2, space="PSUM"))

        # ---- constants into SBUF ----
        qwt_t = const_pool.tile([128, (RNN // 128) * ATT], F32)
        ht_t = const_pool.tile([128, (RNN // 128) * BP], F32)
        for c in range(RNN // 128):
            nc.sync.dma_start(qwt_t[:, ts(c, ATT)], qWT[c * 128:(c + 1) * 128, :])
            nc.sync.dma_start(ht_t[:, ts(c, BP)], hiddenT[c * 128:(c + 1) * 128, :])
        cwr_t = const_pool.tile([NF, CK], F32)
        nc.sync.dma_start(cwr_t[:], cwr)
        locwt_t = const_pool.tile([NF, ATT], F32)
        nc.sync.dma_start(locwt_t[:], locWT)
        vt_t = const_pool.tile([ATT, 1], F32)
        nc.sync.dma_start(vt_t[:], vT)
        maskadd_t = const_pool.tile([BP, T], F32)
        nc.sync.dma_start(maskadd_t[:], maskadd)
        ident_t = const_pool.tile([128, 128], F32)
        masks.make_identity(nc, ident_t[:])

        # ---- pq = query_W @ hidden.T : (ATT=128, BP=16) ----
        ps_pq = ps_misc_pool.tile([ATT, BP], F32, tag="misc")
        for c in range(RNN // 128):
            nc.tensor.matmul(ps_pq[:], qwt_t[:, ts(c, ATT)], ht_t[:, ts(c, BP)],
                             start=(c == 0), stop=(c == RNN // 128 - 1))
        pq_t = const_pool.tile([ATT, BP], F32)
        nc.vector.tensor_copy(pq_t[:], ps_pq[:])

        # ---- CW2T[(c,k), a] = sum_f conv_W[f,(c,k)] loc_W[a,f] : (62, 128) ----
        ps_cw = ps_misc_pool.tile([CK, ATT], F32, tag="misc")
        nc.tensor.matmul(ps_cw[:], cwr_t[:], locwt_t[:], start=True, stop=True)
        cw2t_t = const_pool.tile([CK, ATT], F32)
        nc.vector.tensor_copy(cw2t_t[:], ps_cw[:])

        # ---- phase 1: energies e[b, t] ----
        e_all = const_pool.tile([BP, T], F32)
        for b in range(BP):
            xs_t = xs_pool.tile([CK, T], F32)
            nc.sync.dma_start(xs_t[:], xs[b])
            pm_t = pm_pool.tile([ATT, T], F32)
            nc.sync.dma_start(pm_t[:], pmT[b])
            for c in range(NC4):
                ps_loc = ps_loc_pool.tile([ATT, 512], F32)
                nc.tensor.matmul(ps_loc[:], cw2t_t[:], xs_t[:, ts(c, 512)],
                                 start=True, stop=True)
                s_t = s_pool.tile([ATT, 512], F32)
                nc.vector.tensor_add(s_t[:], ps_loc[:], pm_t[:, ts(c, 512)])
                th_t = th_pool.tile([ATT, 512], F32)
                nc.scalar.activation(th_t[:], s_t[:],
                                     mybir.ActivationFunctionType.Tanh,
                                     bias=pq_t[:, b:b + 1])
                ps_e = ps_e_pool.tile([1, 512], F32)
                nc.tensor.matmul(ps_e[:], vt_t[:], th_t[:], start=True, stop=True)
                e_stage = o_pool.tile([1, 512], F32, tag="estage")
                nc.vector.tensor_copy(e_stage[:], ps_e[:])
                nc.sync.dma_start(e_all[b:b + 1, ts(c, 512)], e_stage[:])

        # ---- softmax over T (batch rows on partitions) ----
        em_t = w_pool.tile([BP, T], F32)
        nc.vector.tensor_add(em_t[:], e_all[:], maskadd_t[:])
        negmax_t = const_pool.tile([BP, 1], F32)
        nc.vector.reduce_max(negmax_t[:], em_t[:], axis=mybir.AxisListType.X,
                             negate=True)
        p_t = w_pool.tile([BP, T], F32)
        rowsum_t = const_pool.tile([BP, 1], F32)
        nc.scalar.activation(p_t[:], em_t[:], mybir.ActivationFunctionType.Exp,
                             bias=negmax_t[:], accum_out=rowsum_t[:])
        rinv_t = const_pool.tile([BP, 1], F32)
        nc.vector.reciprocal(rinv_t[:], rowsum_t[:])
        w_t = w_pool.tile([BP, T], F32)
        nc.vector.tensor_scalar_mul(w_t[:], p_t[:], rinv_t[:])
        nc.sync.dma_start(out_w[:, :], w_t[:])

        # ---- transpose w -> wT (128 t-rows, NT*BP cols) ----
        wT_t = const_pool.tile([128, NT * BP], F32)
        for t in range(NT):
            ps_tr = ps_misc_pool.tile([128, BP], F32, tag="misc")
            nc.tensor.transpose(ps_tr[:], w_t[:, ts(t, 128)], ident_t[:BP, :BP])
            nc.vector.tensor_copy(wT_t[:, ts(t, BP)], ps_tr[:])

        # ---- phase 2: ctx[b] = sum_t w[b,t] * mem[b,t,:] ----
        for b in range(BP):
            ps_c = ps_ctx_pool.tile([1, ENC], F32)
            for g in range(4):
                mem_t = mem_pool.tile([128, 4 * ENC], F32)
                nc.sync.dma_start(
                    mem_t[:].rearrange("p (c d) -> p c d", d=ENC),
                    mem[b, g * 512:(g + 1) * 512, :].rearrange(
                        "(c p) d -> p c d", p=128))
                for c in range(4):
                    t_idx = g * 4 + c
                    nc.tensor.matmul(ps_c[:],
                                     wT_t[:, t_idx * BP + b:t_idx * BP + b + 1],
                                     mem_t[:, ts(c, ENC)],
                                     start=(t_idx == 0), stop=(t_idx == NT - 1))
            ctx_t = o_pool.tile([1, ENC], F32)
            nc.vector.tensor_copy(ctx_t[:], ps_c[:])
            nc.sync.dma_start(out_ctx[b:b + 1, :], ctx_t[:])

    nc.compile()
    return nc


def _marshal(inputs):
    """Full inputs -> per-core in_maps (host-side layout only, no math)."""
    hid = np.ascontiguousarray(np.asarray(inputs["attention_hidden_state"], np.float32))
    memory = np.ascontiguousarray(np.asarray(inputs["memory"], np.float32))
    pm = np.asarray(inputs["processed_memory"], np.float32)
    awc = np.asarray(inputs["attention_weights_cat"], np.float32)
    mask = np.asarray(inputs["mask"])
    qW = np.asarray(inputs["query_W"], np.float32)
    cW = np.asarray(inputs["conv_W"], np.float32)
    lW = np.asarray(inputs["loc_W"], np.float32)
    vW = np.asarray(inputs["v_W"], np.float32)

    hT = np.ascontiguousarray(hid.T)                       # (RNN, B)
    qWT = np.ascontiguousarray(qW.T)                       # (RNN, ATT)
    cwr = np.ascontiguousarray(cW.reshape(NF, CK))         # (32, 62)
    locWT = np.ascontiguousarray(lW.T)                     # (32, 128)
    vT = np.ascontiguousarray(vW.T)                        # (128, 1)

    xp = np.zeros((B, 2, T + 2 * PAD), np.float32)
    xp[:, :, PAD:PAD + T] = awc
    s0, s1, s2 = xp.strides
    xs_view = np.lib.stride_tricks.as_strided(
        xp, shape=(B, 2, KS, T), strides=(s0, s1, s2, s2))
    xs = np.ascontiguousarray(xs_view.reshape(B, CK, T))   # (B, 62, T)

    pmT = np.ascontiguousarray(pm.transpose(0, 2, 1))      # (B, ATT, T)
    maskadd = np.where(mask, np.float32(-1e30), np.float32(0.0)).astype(np.float32)

    in_maps = []
    for c in range(N_CORES):
        sl = slice(c * BP, (c + 1) * BP)
        in_maps.append({
            "hiddenT": np.ascontiguousarray(hT[:, sl]),
            "qWT": qWT,
            "xs": xs[sl],
            "cwr": cwr,
            "locWT": locWT,
            "vT": vT,
            "pmT": pmT[sl],
            "maskadd": np.ascontiguousarray(maskadd[sl]),
            "mem": memory[sl],
        })
    return in_maps


def kernel(**inputs):
    global _NC_CACHE, LAST_RESULT
    if _NC_CACHE is None:
        _NC_CACHE = _build_nc()
    nc = _NC_CACHE
    in_maps = _marshal(inputs)
    res = run_bass_kernel_spmd(nc, in_maps, core_ids=list(range(N_CORES)),
                               trace=_TRACE)
    LAST_RESULT = res
    ctx = np.concatenate([r["out_ctx"] for r in res.results], axis=0)
    w = np.concatenate([r["out_w"] for r in res.results], axis=0)
    return ctx, w


# revision 5
# speedup vs baseline: 1.8362x; 1.8362x over previous
"""Tacotron2 location-sensitive attention on 8 TRN2 NeuronCores.

Data-parallel over batch B=128 -> 16 rows per core; params replicated.
Per core (b = 16 batch rows):
  pq    = hidden @ query_W.T                       (PE, tiny, f32)
  loc   = conv1d(aw_cat) @ loc_W.T  -- folded: CW2[a,(c,k)] = loc_W @ conv_W,
          conv done as one matmul over im2col'd input (host-marshalled)
  e     = v . tanh(pq + loc + pm)                  (PE + DVE + ACT)
  w     = softmax(mask(e))                         (DVE + ACT f32, batch on partitions)
  ctx   = w @ memory                               (PE matvec, streams memory)

Matmul-facing tensors are bf16 (PE fp32 streams at 1/4 rate); softmax and
all accumulations stay f32.  memory/processed_memory/im2col are host-cast
to bf16, halving the dominant DMA traffic.
"""

import os
import numpy as np
from contextlib import ExitStack

import concourse.bass as bass
import concourse.bacc as bacc
import concourse.tile as tile
from concourse import mybir, masks
from concourse.bass_utils import run_bass_kernel_spmd

F32 = mybir.dt.float32
BF16 = mybir.dt.bfloat16
ts = bass.ts

N_CORES = 8
B, T = 128, 2048
BP = B // N_CORES          # 16 batch rows per core
RNN, ATT, ENC = 1024, 128, 512
NF, KS, PAD = 32, 31, 15
CK = 2 * KS                # 62 im2col rows
NT = T // 128              # 16 T-chunks of 128

_TRACE = os.environ.get("BASS_KERNEL_TRACE", "0") == "1"
LAST_RESULT = None
_NC_CACHE = None


def _build_nc():
    nc = bacc.Bacc("TRN2", target_bir_lowering=False, debug=False,
                   num_devices=N_CORES)

    hiddenT = nc.dram_tensor("hiddenT", [RNN, BP], F32, kind="ExternalInput").ap()
    qWT = nc.dram_tensor("qWT", [RNN, ATT], F32, kind="ExternalInput").ap()
    xs = nc.dram_tensor("xs", [BP, CK, T], BF16, kind="ExternalInput").ap()
    cwr = nc.dram_tensor("cwr", [NF, CK], F32, kind="ExternalInput").ap()
    locWT = nc.dram_tensor("locWT", [NF, ATT], F32, kind="ExternalInput").ap()
    vT = nc.dram_tensor("vT", [ATT, 1], BF16, kind="ExternalInput").ap()
    pmT = nc.dram_tensor("pmT", [BP, ATT, T], BF16, kind="ExternalInput").ap()
    maskadd = nc.dram_tensor("maskadd", [BP, T], F32, kind="ExternalInput").ap()
    mem = nc.dram_tensor("mem", [BP, T, ENC], BF16, kind="ExternalInput").ap()

    out_ctx = nc.dram_tensor("out_ctx", [BP, ENC], F32, kind="ExternalOutput").ap()
    out_w = nc.dram_tensor("out_w", [BP, T], F32, kind="ExternalOutput").ap()

    with tile.TileContext(nc) as tc, ExitStack() as ctx:
        const_pool = ctx.enter_context(tc.tile_pool(name="const", bufs=1))
        xs_pool = ctx.enter_context(tc.tile_pool(name="xs", bufs=3))
        pm_pool = ctx.enter_context(tc.tile_pool(name="pm", bufs=3))
        s_pool = ctx.enter_context(tc.tile_pool(name="s", bufs=4))
        th_pool = ctx.enter_context(tc.tile_pool(name="th", bufs=4))
        w_pool = ctx.enter_context(tc.tile_pool(name="w", bufs=1))
        mem_pool = ctx.enter_context(tc.tile_pool(name="mem", bufs=6))
        o_pool = ctx.enter_context(tc.tile_pool(name="o", bufs=4))
        ps_loc_pool = ctx.enter_context(tc.tile_pool(name="psloc", bufs=2, space="PSUM"))
        ps_e_pool = ctx.enter_context(tc.tile_pool(name="pse", bufs=2, space="PSUM"))
        ps_misc_pool = ctx.enter_context(tc.tile_pool(name="psmisc", bufs=2, space="PSUM"))
        ps_ctx_pool = ctx.enter_context(tc.tile_pool(name="psctx", bufs=2, space="PSUM"))

        # ---- constants into SBUF ----
        qwt_t = const_pool.tile([128, (RNN // 128) * ATT], F32)
        ht_t = const_pool.tile([128, (RNN // 128) * BP], F32)
        for c in range(RNN // 128):
            nc.sync.dma_start(qwt_t[:, ts(c, ATT)], qWT[c * 128:(c + 1) * 128, :])
            nc.sync.dma_start(ht_t[:, ts(c, BP)], hiddenT[c * 128:(c + 1) * 128, :])
        cwr_t = const_pool.tile([NF, CK], F32)
        nc.sync.dma_start(cwr_t[:], cwr)
        locwt_t = const_pool.tile([NF, ATT], F32)
        nc.sync.dma_start(locwt_t[:], locWT)
        vt_t = const_pool.tile([ATT, 1], BF16)
        nc.sync.dma_start(vt_t[:], vT)
        maskadd_t = const_pool.tile([BP, T], F32)
        nc.sync.dma_start(maskadd_t[:], maskadd)
        ident_t = const_pool.tile([128, 128], BF16)
        masks.make_identity(nc, ident_t[:])

        # ---- pq = query_W @ hidden.T : (ATT=128, BP=16), f32 ----
        ps_pq = ps_misc_pool.tile([ATT, BP], F32, tag="misc")
        for c in range(RNN // 128):
            nc.tensor.matmul(ps_pq[:], qwt_t[:, ts(c, ATT)], ht_t[:, ts(c, BP)],
                             start=(c == 0), stop=(c == RNN // 128 - 1))
        pq_t = const_pool.tile([ATT, BP], F32)
        nc.vector.tensor_copy(pq_t[:], ps_pq[:])

        # ---- CW2T[(c,k), a] = sum_f conv_W[f,(c,k)] loc_W[a,f] : (62, 128) ----
        ps_cw = ps_misc_pool.tile([CK, ATT], F32, tag="misc")
        nc.tensor.matmul(ps_cw[:], cwr_t[:], locwt_t[:], start=True, stop=True)
        cw2t_t = const_pool.tile([CK, ATT], BF16)
        nc.vector.tensor_copy(cw2t_t[:], ps_cw[:])

        # ---- phase 1: energies e[b, t] ----
        e_all = const_pool.tile([BP, T], F32)
        for b in range(BP):
            xs_t = xs_pool.tile([CK, T], BF16)
            nc.sync.dma_start(xs_t[:], xs[b])
            pm_t = pm_pool.tile([ATT, T], BF16)
            nc.sync.dma_start(pm_t[:], pmT[b])
            for c in range(T // 512):
                ps_loc = ps_loc_pool.tile([ATT, 512], F32)
                nc.tensor.matmul(ps_loc[:], cw2t_t[:], xs_t[:, ts(c, 512)],
                                 start=True, stop=True)
                s_t = s_pool.tile([ATT, 512], BF16)
                nc.vector.tensor_add(s_t[:], ps_loc[:], pm_t[:, ts(c, 512)])
                th_t = th_pool.tile([ATT, 512], BF16)
                nc.scalar.activation(th_t[:], s_t[:],
                                     mybir.ActivationFunctionType.Tanh,
                                     bias=pq_t[:, b:b + 1])
                ps_e = ps_e_pool.tile([1, 512], F32)
                nc.tensor.matmul(ps_e[:], vt_t[:], th_t[:], start=True, stop=True)
                e_stage = o_pool.tile([1, 512], F32, tag="estage")
                nc.scalar.copy(e_stage[:], ps_e[:])
                nc.sync.dma_start(e_all[b:b + 1, ts(c, 512)], e_stage[:])

        # ---- softmax over T (batch rows on partitions), f32 ----
        em_t = w_pool.tile([BP, T], F32)
        nc.vector.tensor_add(em_t[:], e_all[:], maskadd_t[:])
        negmax_t = const_pool.tile([BP, 1], F32)
        nc.vector.reduce_max(negmax_t[:], em_t[:], axis=mybir.AxisListType.X,
                             negate=True)
        p_t = w_pool.tile([BP, T], F32)
        rowsum_t = const_pool.tile([BP, 1], F32)
        nc.scalar.activation(p_t[:], em_t[:], mybir.ActivationFunctionType.Exp,
                             bias=negmax_t[:], accum_out=rowsum_t[:])
        rinv_t = const_pool.tile([BP, 1], F32)
        nc.vector.reciprocal(rinv_t[:], rowsum_t[:])
        w_t = w_pool.tile([BP, T], F32)
        nc.vector.tensor_scalar_mul(w_t[:], p_t[:], rinv_t[:])
        nc.sync.dma_start(out_w[:, :], w_t[:])
        w_bf = w_pool.tile([BP, T], BF16)
        nc.vector.tensor_copy(w_bf[:], w_t[:])

        # ---- transpose w -> wT (128 t-rows, NT*BP cols), bf16 ----
        wT_t = const_pool.tile([128, NT * BP], BF16)
        for t in range(NT):
            ps_tr = ps_misc_pool.tile([128, BP], BF16, tag="misc")
            nc.tensor.transpose(ps_tr[:], w_bf[:, ts(t, 128)], ident_t[:BP, :BP])
            nc.vector.tensor_copy(wT_t[:, ts(t, BP)], ps_tr[:])

        # ---- phase 2: ctx[b] = sum_t w[b,t] * mem[b,t,:] ----
        for b in range(BP):
            ps_c = ps_ctx_pool.tile([1, ENC], F32)
            for h in range(2):
                mem_t = mem_pool.tile([128, 8 * ENC], BF16)
                nc.sync.dma_start(
                    mem_t[:].rearrange("p (c d) -> p c d", d=ENC),
                    mem[b, h * 1024:(h + 1) * 1024, :].rearrange(
                        "(c p) d -> p c d", p=128))
                for c in range(8):
                    t_idx = h * 8 + c
                    nc.tensor.matmul(ps_c[:],
                                     wT_t[:, t_idx * BP + b:t_idx * BP + b + 1],
                                     mem_t[:, ts(c, ENC)],
                                     start=(t_idx == 0), stop=(t_idx == NT - 1))
            ctx_t = o_pool.tile([1, ENC], F32)
            nc.vector.tensor_copy(ctx_t[:], ps_c[:])
            nc.sync.dma_start(out_ctx[b:b + 1, :], ctx_t[:])

    nc.compile()
    return nc


def _marshal(inputs):
    """Full inputs -> per-core in_maps (host-side layout/dtype only)."""
    from ml_dtypes import bfloat16

    hid = np.ascontiguousarray(np.asarray(inputs["attention_hidden_state"], np.float32))
    memory = np.asarray(inputs["memory"], np.float32)
    pm = np.asarray(inputs["processed_memory"], np.float32)
    awc = np.asarray(inputs["attention_weights_cat"], np.float32)
    mask = np.asarray(inputs["mask"])
    qW = np.asarray(inputs["query_W"], np.float32)
    cW = np.asarray(inputs["conv_W"], np.float32)
    lW = np.asarray(inputs["loc_W"], np.float32)
    vW = np.asarray(inputs["v_W"], np.float32)

    hT = np.ascontiguousarray(hid.T)                       # (RNN, B)
    qWT = np.ascontiguousarray(qW.T)                       # (RNN, ATT)
    cwr = np.ascontiguousarray(cW.reshape(NF, CK))         # (32, 62)
    locWT = np.ascontiguousarray(lW.T)                     # (32, 128)
    vT = np.ascontiguousarray(vW.T).astype(bfloat16)       # (128, 1)

    xp = np.zeros((B, 2, T + 2 * PAD), np.float32)
    xp[:, :, PAD:PAD + T] = awc
    s0, s1, s2 = xp.strides
    xs_view = np.lib.stride_tricks.as_strided(
        xp, shape=(B, 2, KS, T), strides=(s0, s1, s2, s2))
    xs = np.ascontiguousarray(xs_view.reshape(B, CK, T)).astype(bfloat16)

    pmT = np.ascontiguousarray(pm.transpose(0, 2, 1)).astype(bfloat16)
    maskadd = np.where(mask, np.float32(-1e30), np.float32(0.0)).astype(np.float32)
    mem_bf = memory.astype(bfloat16)

    in_maps = []
    for c in range(N_CORES):
        sl = slice(c * BP, (c + 1) * BP)
        in_maps.append({
            "hiddenT": np.ascontiguousarray(hT[:, sl]),
            "qWT": qWT,
            "xs": xs[sl],
            "cwr": cwr,
            "locWT": locWT,
            "vT": vT,
            "pmT": pmT[sl],
            "maskadd": np.ascontiguousarray(maskadd[sl]),
            "mem": np.ascontiguousarray(mem_bf[sl]),
        })
    return in_maps


def kernel(**inputs):
    global _NC_CACHE, LAST_RESULT
    if _NC_CACHE is None:
        _NC_CACHE = _build_nc()
    nc = _NC_CACHE
    in_maps = _marshal(inputs)
    res = run_bass_kernel_spmd(nc, in_maps, core_ids=list(range(N_CORES)),
                               trace=_TRACE)
    LAST_RESULT = res
    ctx = np.concatenate([r["out_ctx"] for r in res.results], axis=0)
    w = np.concatenate([r["out_w"] for r in res.results], axis=0)
    return ctx, w


# revision 9
# speedup vs baseline: 2.3087x; 1.2574x over previous
"""Tacotron2 location-sensitive attention on 8 TRN2 NeuronCores.

Data-parallel over batch B=128 -> 16 rows per core; params replicated.
Per core (b = 16 batch rows):
  pq    = hidden @ query_W.T                       (PE, tiny, f32)
  loc   = conv1d(aw_cat) @ loc_W.T  -- folded: CW2[a,(c,k)] = loc_W @ conv_W,
          conv done as one matmul over im2col'd input (host-marshalled)
  e     = v . tanh(pq + loc + pm)                  (PE + DVE + ACT)
  w     = softmax(mask(e))                         (DVE + ACT f32, batch on partitions)
  ctx   = w @ memory                               (PE matvec, streams memory)

Matmul-facing tensors are bf16 (PE fp32 streams at 1/4 rate); softmax and
all accumulations stay f32.  memory/processed_memory/im2col are host-cast
to bf16, halving the dominant DMA traffic.
"""

import os
import numpy as np
from contextlib import ExitStack

import concourse.bass as bass
import concourse.bacc as bacc
import concourse.tile as tile
from concourse import mybir, masks
from concourse.bass_utils import run_bass_kernel_spmd

F32 = mybir.dt.float32
BF16 = mybir.dt.bfloat16
ts = bass.ts

N_CORES = 8
B, T = 128, 2048
BP = B // N_CORES          # 16 batch rows per core
RNN, ATT, ENC = 1024, 128, 512
NF, KS, PAD = 32, 31, 15
CK = 2 * KS                # 62 im2col rows
NT = T // 128              # 16 T-chunks of 128

_TRACE = os.environ.get("BASS_KERNEL_TRACE", "0") == "1"
LAST_RESULT = None
_NC_CACHE = None


def _build_nc():
    nc = bacc.Bacc("TRN2", target_bir_lowering=False, debug=False,
                   num_devices=N_CORES)

    hiddenT = nc.dram_tensor("hiddenT", [RNN, BP], F32, kind="ExternalInput").ap()
    qWT = nc.dram_tensor("qWT", [RNN, ATT], F32, kind="ExternalInput").ap()
    xs = nc.dram_tensor("xs", [BP, CK, T], BF16, kind="ExternalInput").ap()
    cwr = nc.dram_tensor("cwr", [NF, CK], F32, kind="ExternalInput").ap()
    locWT = nc.dram_tensor("locWT", [NF, ATT], F32, kind="ExternalInput").ap()
    vT = nc.dram_tensor("vT", [ATT, 1], BF16, kind="ExternalInput").ap()
    pmT = nc.dram_tensor("pmT", [BP, ATT, T], BF16, kind="ExternalInput").ap()
    maskadd = nc.dram_tensor("maskadd", [BP, T], F32, kind="ExternalInput").ap()
    mem = nc.dram_tensor("mem", [BP, T, ENC], BF16, kind="ExternalInput").ap()

    out_ctx = nc.dram_tensor("out_ctx", [BP, ENC], F32, kind="ExternalOutput").ap()
    out_w = nc.dram_tensor("out_w", [BP, T], F32, kind="ExternalOutput").ap()

    with tile.TileContext(nc) as tc, ExitStack() as ctx:
        const_pool = ctx.enter_context(tc.tile_pool(name="const", bufs=1))
        xs_pool = ctx.enter_context(tc.tile_pool(name="xs", bufs=3))
        pm_pool = ctx.enter_context(tc.tile_pool(name="pm", bufs=3))
        s_pool = ctx.enter_context(tc.tile_pool(name="s", bufs=3))
        th_pool = ctx.enter_context(tc.tile_pool(name="th", bufs=3))
        w_pool = ctx.enter_context(tc.tile_pool(name="w", bufs=1))
        mem_pool = ctx.enter_context(tc.tile_pool(name="mem", bufs=10))
        o_pool = ctx.enter_context(tc.tile_pool(name="o", bufs=4))
        ps_loc_pool = ctx.enter_context(tc.tile_pool(name="psloc", bufs=2, space="PSUM"))
        ps_e_pool = ctx.enter_context(tc.tile_pool(name="pse", bufs=2, space="PSUM"))
        ps_mc_pool = ctx.enter_context(tc.tile_pool(name="psmc", bufs=2, space="PSUM"))

        # ---- constants into SBUF ----
        qwt_t = const_pool.tile([128, (RNN // 128) * ATT], F32)
        ht_t = const_pool.tile([128, (RNN // 128) * BP], F32)
        for c in range(RNN // 128):
            nc.sync.dma_start(qwt_t[:, ts(c, ATT)], qWT[c * 128:(c + 1) * 128, :])
            nc.sync.dma_start(ht_t[:, ts(c, BP)], hiddenT[c * 128:(c + 1) * 128, :])
        cwr_t = const_pool.tile([NF, CK], F32)
        nc.sync.dma_start(cwr_t[:], cwr)
        locwt_t = const_pool.tile([NF, ATT], F32)
        nc.sync.dma_start(locwt_t[:], locWT)
        vt_t = const_pool.tile([ATT, 1], BF16)
        nc.sync.dma_start(vt_t[:], vT)
        maskadd_t = const_pool.tile([BP, T], F32)
        nc.sync.dma_start(maskadd_t[:], maskadd)
        ident_t = const_pool.tile([128, 128], BF16)
        masks.make_identity(nc, ident_t[:])

        # ---- pq = query_W @ hidden.T : (ATT=128, BP=16), f32 ----
        ps_pq = ps_mc_pool.tile([ATT, BP], F32, tag="mc")
        for c in range(RNN // 128):
            nc.tensor.matmul(ps_pq[:], qwt_t[:, ts(c, ATT)], ht_t[:, ts(c, BP)],
                             start=(c == 0), stop=(c == RNN // 128 - 1))
        pq_t = const_pool.tile([ATT, BP], F32)
        nc.vector.tensor_copy(pq_t[:], ps_pq[:])

        # ---- CW2T[(c,k), a] = sum_f conv_W[f,(c,k)] loc_W[a,f] : (62, 128) ----
        ps_cw = ps_mc_pool.tile([CK, ATT], F32, tag="mc")
        nc.tensor.matmul(ps_cw[:], cwr_t[:], locwt_t[:], start=True, stop=True)
        cw2t_t = const_pool.tile([CK, ATT], BF16)
        nc.vector.tensor_copy(cw2t_t[:], ps_cw[:])

        # ---- phase 1: energies e[b, t] ----
        e_all = const_pool.tile([BP, T], F32)
        for b in range(BP):
            xs_t = xs_pool.tile([CK, T], BF16)
            nc.gpsimd.dma_start(xs_t[:], xs[b])
            pm_t = pm_pool.tile([ATT, T], BF16)
            nc.gpsimd.dma_start(pm_t[:], pmT[b])
            e_stage = o_pool.tile([1, T], F32, tag="estage")
            for h in range(2):
                ps_loc = ps_loc_pool.tile([ATT, 1024], F32)
                for c in range(2):
                    nc.tensor.matmul(ps_loc[:, ts(c, 512)], cw2t_t[:],
                                     xs_t[:, ts(h * 2 + c, 512)],
                                     start=True, stop=True)
                s_t = s_pool.tile([ATT, 1024], BF16)
                nc.vector.tensor_add(s_t[:], ps_loc[:], pm_t[:, ts(h, 1024)])
                th_t = th_pool.tile([ATT, 1024], BF16)
                nc.scalar.activation(th_t[:], s_t[:],
                                     mybir.ActivationFunctionType.Tanh,
                                     bias=pq_t[:, b:b + 1])
                for c in range(2):
                    ps_e = ps_e_pool.tile([1, 512], F32)
                    nc.tensor.matmul(ps_e[:], vt_t[:], th_t[:, ts(c, 512)],
                                     start=True, stop=True)
                    nc.any.tensor_copy(out=e_stage[:, ts(h * 2 + c, 512)],
                                       in_=ps_e[:])
            nc.scalar.dma_start(e_all[b:b + 1, :], e_stage[:])

        # ---- softmax over T (batch rows on partitions), f32 ----
        em_t = w_pool.tile([BP, T], F32)
        nc.vector.tensor_add(em_t[:], e_all[:], maskadd_t[:])
        negmax_t = const_pool.tile([BP, 1], F32)
        nc.vector.reduce_max(negmax_t[:], em_t[:], axis=mybir.AxisListType.X,
                             negate=True)
        p_t = w_pool.tile([BP, T], F32)
        rowsum_t = const_pool.tile([BP, 1], F32)
        nc.scalar.activation(p_t[:], em_t[:], mybir.ActivationFunctionType.Exp,
                             bias=negmax_t[:], accum_out=rowsum_t[:])
        rinv_t = const_pool.tile([BP, 1], F32)
        nc.vector.reciprocal(rinv_t[:], rowsum_t[:])
        w_t = w_pool.tile([BP, T], F32)
        nc.vector.tensor_scalar_mul(w_t[:], p_t[:], rinv_t[:])
        nc.sync.dma_start(out_w[:, :], w_t[:])
        w_bf = w_pool.tile([BP, T], BF16)
        nc.vector.tensor_copy(w_bf[:], w_t[:])

        # ---- transpose w -> wT (128 t-rows, NT*BP cols), bf16 ----
        wT_t = const_pool.tile([128, NT * BP], BF16)
        for t in range(NT):
            ps_tr = ps_mc_pool.tile([128, BP], BF16, tag="mc")
            nc.tensor.transpose(ps_tr[:], w_bf[:, ts(t, 128)], ident_t[:BP, :BP])
            nc.vector.tensor_copy(wT_t[:, ts(t, BP)], ps_tr[:])

        # ---- phase 2: ctx[b] = sum_t w[b,t] * mem[b,t,:] ----
        for b in range(BP):
            ps_c = ps_mc_pool.tile([1, ENC], F32, tag="mc")
            for h in range(2):
                mem_t = mem_pool.tile([128, 8 * ENC], BF16)
                nc.sync.dma_start(
                    mem_t[:].rearrange("p (c d) -> p c d", d=ENC),
                    mem[b, h * 1024:(h + 1) * 1024, :].rearrange(
                        "(c p) d -> p c d", p=128))
                for c in range(8):
                    t_idx = h * 8 + c
                    nc.tensor.matmul(ps_c[:],
                                     wT_t[:, t_idx * BP + b:t_idx * BP + b + 1],
                                     mem_t[:, ts(c, ENC)],
                                     start=(t_idx == 0), stop=(t_idx == NT - 1))
            ctx_t = o_pool.tile([1, ENC], F32)
            nc.vector.tensor_copy(ctx_t[:], ps_c[:])
            nc.scalar.dma_start(out_ctx[b:b + 1, :], ctx_t[:])

    nc.compile()
    return nc


def _marshal(inputs):
    """Full inputs -> per-core in_maps (host-side layout/dtype only)."""
    from ml_dtypes import bfloat16

    hid = np.ascontiguousarray(np.asarray(inputs["attention_hidden_state"], np.float32))
    memory = np.asarray(inputs["memory"], np.float32)
    pm = np.asarray(inputs["processed_memory"], np.float32)
    awc = np.asarray(inputs["attention_weights_cat"], np.float32)
    mask = np.asarray(inputs["mask"])
    qW = np.asarray(inputs["query_W"], np.float32)
    cW = np.asarray(inputs["conv_W"], np.float32)
    lW = np.asarray(inputs["loc_W"], np.float32)
    vW = np.asarray(inputs["v_W"], np.float32)

    hT = np.ascontiguousarray(hid.T)                       # (RNN, B)
    qWT = np.ascontiguousarray(qW.T)                       # (RNN, ATT)
    cwr = np.ascontiguousarray(cW.reshape(NF, CK))         # (32, 62)
    locWT = np.ascontiguousarray(lW.T)                     # (32, 128)
    vT = np.ascontiguousarray(vW.T).astype(bfloat16)       # (128, 1)

    xp = np.zeros((B, 2, T + 2 * PAD), np.float32)
    xp[:, :, PAD:PAD + T] = awc
    s0, s1, s2 = xp.strides
    xs_view = np.lib.stride_tricks.as_strided(
        xp, shape=(B, 2, KS, T), strides=(s0, s1, s2, s2))
    xs = np.ascontiguousarray(xs_view.reshape(B, CK, T)).astype(bfloat16)

    pmT = np.ascontiguousarray(pm.transpose(0, 2, 1)).astype(bfloat16)
    maskadd = np.where(mask, np.float32(-1e30), np.float32(0.0)).astype(np.float32)
    mem_bf = memory.astype(bfloat16)

    in_maps = []
    for c in range(N_CORES):
        sl = slice(c * BP, (c + 1) * BP)
        in_maps.append({
            "hiddenT": np.ascontiguousarray(hT[:, sl]),
            "qWT": qWT,
            "xs": xs[sl],
            "cwr": cwr,
            "locWT": locWT,
            "vT": vT,
            "pmT": pmT[sl],
            "maskadd": np.ascontiguousarray(maskadd[sl]),
            "mem": np.ascontiguousarray(mem_bf[sl]),
        })
    return in_maps


def kernel(**inputs):
    global _NC_CACHE, LAST_RESULT
    if _NC_CACHE is None:
        _NC_CACHE = _build_nc()
    nc = _NC_CACHE
    in_maps = _marshal(inputs)
    res = run_bass_kernel_spmd(nc, in_maps, core_ids=list(range(N_CORES)),
                               trace=_TRACE)
    LAST_RESULT = res
    ctx = np.concatenate([r["out_ctx"] for r in res.results], axis=0)
    w = np.concatenate([r["out_w"] for r in res.results], axis=0)
    return ctx, w
